# revision 2
# baseline (speedup 1.0000x reference)
"""Trainium2 Bass kernel for the 21-joint hand-graph message-passing MLP.

Math (per sample b, per target joint t with neighbor list S_t of length n):
    g   = concat(x[b, S_t[0]], ..., x[b, S_t[n-1]])          # [n*64]
    h1  = relu(g @ W1_t + b1_t)                              # [128]
    h2  = relu(h1 @ W2_t + b2_t)                             # [128]
    out[b, t] = h2 @ W3_t + b3_t                             # [64]

Strategy (pure data parallel over 8 NeuronCores, B=65536 -> 8192/core):
  - x is loaded batch-major with an fp32->bf16 cast DMA (SWDGE), then
    DMA-xbar-transposed (bf16-only HW path) into feature-major tiles
    xT[p] = [128 feats of node pair (2p, 2p+1), batch] so every matmul can
    contract features along the partition dim.
  - L1/L2 run weight-stationary: psum[h, batch] = W.T @ act, relu+bias is
    fused into the PSUM->SBUF evacuation (ScalarE activation / VectorE
    tensor_scalar, split across both engines since they are the only PSUM
    readers).
  - L3 runs activation-stationary (lhsT = h2 chunk [128 feats, 128 batch],
    rhs = W3 [128, 64]) so the output lands batch-major in PSUM and the
    final store to out[B, 21, 64] is a plain full-rate DMA, with no output
    transpose. b3 (a free-dim bias in this orientation) is pre-filled into
    PSUM with a K=1 ones-matmul that the L3 matmuls accumulate onto.
  - W1 rows are permuted host-side so each 128-row contraction chunk is
    either one resident node pair tile or a 64-row single placed at the
    partition base (0/64) matching its slot, letting the PE run two K=64
    matmuls concurrently in disjoint row-groups.
"""

import os
import numpy as np
import ml_dtypes

B, J, D, H1, H2 = 65536, 21, 64, 128, 128
NCORES = 8
BC = B // NCORES          # 8192 samples per core
TILE = 1024               # batch tile (2 PSUM banks wide in fp32)
NTILES = BC // TILE       # 8
NCHUNK = TILE // 128      # 8 L3 batch chunks of 128 per tile

FINGER_BASE = [4 * f + 1 for f in range(5)]
NEIGH = {
    6: [[0, 1, 5, 9, 13, 17]],
    5: [[0, 5, 6, 1, 9], [0, 9, 10, 5, 13], [0, 13, 14, 9, 17]],
    4: [[0, 1, 2, 5], [0, 17, 18, 13]],
    3: [r for b in FINGER_BASE for r in ([b, b + 1, b + 2], [b + 1, b + 2, b + 3])],
    2: [[b + 2, b + 3] for b in FINGER_BASE],
}
OUT = {
    6: [0],
    5: [5, 9, 13],
    4: [1, 17],
    3: [j for b in FINGER_BASE for j in (b + 1, b + 2)],
    2: [b + 3 for b in FINGER_BASE],
}
GROUPS = [6, 5, 4, 3, 2]

# target t -> (n, row index within its group, neighbor list)
TARGET = {}
for n in GROUPS:
    for row, t in enumerate(OUT[n]):
        TARGET[t] = (n, row, list(NEIGH[n][row]))

# xT tile map: node j lives in tile j//2, slot j%2 (partition base 64*(j%2)).
# Exception: tile 10 is built from the transpose of DRAM columns of nodes
# (19, 20), so node 20 lives at tile 10 slot 1, and node 19 is duplicated at
# tile 10 slot 0 (primary copy at tile 9 slot 1).
NPAIRS = 11


def node_slot(j):
    if j == 20:
        return (10, 1)
    return (j // 2, j % 2)


def build_chunk_plan():
    """Per target, split the neighbor positions into contraction chunks.

    chunk = dict(tile, slots) with slots = (pos_or_None for tile half 0,
    pos_or_None for tile half 1); position i covers W1 rows 64*i : 64*i+64.
    Every chunk is a K=128 matmul over one full xT pair tile; a neighbor that
    has no partner in its tile gets the other 64 lhsT rows zero-filled.
    (Two accumulating matmuls in disjoint PE row groups targeting the same
    PSUM bank run concurrently and fault the device, and a K=64 matmul costs
    the same N cycles as K=128 anyway, so zero-padding is free.)
    """
    plan = {}
    for t in range(21):
        n, _, S = TARGET[t]
        used = [False] * n
        chunks = []
        # genuine pairs: nodes {2p, 2p+1} both present -> tile p; {19,20} -> tile 10
        for i in range(n):
            if used[i]:
                continue
            for k in range(n):
                if used[k] or k == i:
                    continue
                a, b = S[i], S[k]
                lo, hi = min(a, b), max(a, b)
                if (lo % 2 == 0 and hi == lo + 1) or (lo, hi) == (19, 20):
                    tile_idx = 10 if (lo, hi) == (19, 20) else lo // 2
                    pi, pk = (i, k) if a == lo else (k, i)
                    chunks.append(dict(tile=tile_idx, slots=(pi, pk)))
                    used[i] = used[k] = True
                    break
        for i in range(n):
            if not used[i]:
                tile_idx, half = node_slot(S[i])
                slots = (i, None) if half == 0 else (None, i)
                chunks.append(dict(tile=tile_idx, slots=slots))
        plan[t] = chunks
    return plan


CHUNK_PLAN = build_chunk_plan()
TOTAL_CHUNKS = sum(len(v) for v in CHUNK_PLAN.values())

# L3 node groups sharing one PSUM bank (64 fp32 of output each)
L3_GROUPS = [list(range(0, 8)), list(range(8, 16)), list(range(16, 21))]


def pack_weights(inputs):
    """Host-side prep: permute/pack all weights into a handful of flat arrays."""
    bf16 = ml_dtypes.bfloat16
    w1p = np.zeros((128, 128 * TOTAL_CHUNKS), np.float32)
    col = 0
    chunk_cols = {}
    for t in range(21):
        n, row, S = TARGET[t]
        W1 = np.asarray(inputs[f"w1_g{n}"][row], np.float32)  # [n*64, 128]
        for ci, ch in enumerate(CHUNK_PLAN[t]):
            for half, pos in enumerate(ch["slots"]):
                if pos is not None:
                    w1p[64 * half:64 * half + 64, col:col + 128] = \
                        W1[64 * pos:64 * pos + 64]
            chunk_cols[(t, ci)] = col
            col += 128
    w2p = np.zeros((128, 128 * 21), np.float32)
    w3p = np.zeros((128, 64 * 21), np.float32)
    b1p = np.zeros((128, 21), np.float32)
    b2p = np.zeros((128, 21), np.float32)
    b3p = np.zeros((1, 64 * 21), np.float32)
    for t in range(21):
        n, row, _ = TARGET[t]
        w2p[:, 128 * t:128 * (t + 1)] = np.asarray(inputs[f"w2_g{n}"][row])
        w3p[:, 64 * t:64 * (t + 1)] = np.asarray(inputs[f"w3_g{n}"][row])
        b1p[:, t] = np.asarray(inputs[f"b1_g{n}"][row])
        b2p[:, t] = np.asarray(inputs[f"b2_g{n}"][row])
        b3p[0, 64 * t:64 * (t + 1)] = np.asarray(inputs[f"b3_g{n}"][row])
    return dict(
        w1p=w1p.astype(bf16), w2p=w2p.astype(bf16), w3p=w3p.astype(bf16),
        b1p=b1p, b2p=b2p, b3p=b3p.astype(bf16),
    ), chunk_cols


def numpy_emulate(inputs, x):
    """Bit-layout-faithful numpy model of what the HW kernel computes (minus
    PSUM rounding): used to validate the chunk plan / packing offline."""
    bf16 = ml_dtypes.bfloat16
    packed, chunk_cols = pack_weights(inputs)
    xb = x.astype(bf16)  # [Bn, 21, 64]
    Bn = x.shape[0]
    xT = {}
    for p in range(10):
        xT[p] = np.concatenate([xb[:, 2 * p], xb[:, 2 * p + 1]], 1).T  # [128, Bn]
    xT[10] = np.concatenate([xb[:, 19], xb[:, 20]], 1).T
    out = np.zeros((Bn, 21, 64), np.float32)
    for t in range(21):
        psum1 = np.zeros((128, Bn), np.float32)
        for ci, ch in enumerate(CHUNK_PLAN[t]):
            col = chunk_cols[(t, ci)]
            lhsT = packed["w1p"][:, col:col + 128].astype(np.float32)
            rhs = xT[ch["tile"]].astype(np.float32)
            psum1 += lhsT.T @ rhs
        h1 = np.maximum(psum1 + packed["b1p"][:, t:t + 1], 0).astype(bf16)
        w2 = packed["w2p"][:, 128 * t:128 * (t + 1)].astype(np.float32)
        psum2 = w2.T @ h1.astype(np.float32)
        h2 = np.maximum(psum2 + packed["b2p"][:, t:t + 1], 0).astype(bf16)
        w3 = packed["w3p"][:, 64 * t:64 * (t + 1)].astype(np.float32)
        b3 = packed["b3p"][0, 64 * t:64 * t + 64].astype(np.float32)
        out[:, t] = (h2.astype(np.float32).T @ w3) + b3[None, :]
    return out


# ---------------------------------------------------------------------------
# Bass kernel
# ---------------------------------------------------------------------------

def build_bass_kernel():
    import concourse.bass as bass
    import concourse.tile as tile
    from concourse import bacc, mybir

    bf16 = mybir.dt.bfloat16
    f32 = mybir.dt.float32
    Relu = mybir.ActivationFunctionType.Relu
    Alu = mybir.AluOpType

    nc = bacc.Bacc("TRN2", target_bir_lowering=False, debug=False,
                   num_devices=NCORES)
    x_dram = nc.dram_tensor("x", [BC, J, D], f32, kind="ExternalInput").ap()
    out_dram = nc.dram_tensor("out", [BC, J, D], f32, kind="ExternalOutput").ap()
    w1_dram = nc.dram_tensor("w1p", [128, 128 * TOTAL_CHUNKS], bf16,
                             kind="ExternalInput").ap()
    w2_dram = nc.dram_tensor("w2p", [128, 128 * 21], bf16, kind="ExternalInput").ap()
    w3_dram = nc.dram_tensor("w3p", [128, 64 * 21], bf16, kind="ExternalInput").ap()
    b1_dram = nc.dram_tensor("b1p", [128, 21], f32, kind="ExternalInput").ap()
    b2_dram = nc.dram_tensor("b2p", [128, 21], f32, kind="ExternalInput").ap()
    b3_dram = nc.dram_tensor("b3p", [1, 64 * 21], bf16, kind="ExternalInput").ap()

    x_flat = x_dram.rearrange("b t d -> b (t d)")      # [BC, 1344]
    out_flat = out_dram.rearrange("b t d -> b (t d)")

    with tile.TileContext(nc) as tc:
        with (
            tc.tile_pool(name="wpool", bufs=1) as wpool,
            tc.tile_pool(name="xbp", bufs=3) as xbp,
            tc.tile_pool(name="xtp", bufs=2) as xtp,
            tc.tile_pool(name="actp", bufs=2) as actp,
            tc.tile_pool(name="h2p", bufs=1) as h2p,
            tc.tile_pool(name="stgp", bufs=1) as stgp,
            tc.tile_pool(name="ps1", bufs=2, space="PSUM") as ps1,
            tc.tile_pool(name="ps2", bufs=1, space="PSUM") as ps2,
            tc.tile_pool(name="ps3", bufs=2, space="PSUM") as ps3,
            tc.tile_pool(name="dramp", bufs=1, space="DRAM") as dramp,
        ):
            # bf16 copy of this core's x in DRAM: written batch-major by the
            # cast pre-pass, re-read column-wise by the xbar transposes (a
            # DMA_TRANSPOSE costs ~1.3us on the Sync sequencer regardless of
            # size, so one [1024,128] DRAM-source transpose replaces eight
            # [128,128] SBUF-source ones)
            xbf = dramp.tile([BC, J * D], bf16, name="xbf")
            w1s = wpool.tile([128, 128 * TOTAL_CHUNKS], bf16, name="w1s")
            w2s = wpool.tile([128, 128 * 21], bf16, name="w2s")
            w3s = wpool.tile([128, 64 * 21], bf16, name="w3s")
            b1s = wpool.tile([128, 21], f32, name="b1s")
            b2s = wpool.tile([128, 21], f32, name="b2s")
            b3s = wpool.tile([1, 64 * 21], bf16, name="b3s")
            ones = wpool.tile([1, 128], bf16, name="ones")
            nc.sync.dma_start(w1s[:], w1_dram)
            nc.sync.dma_start(w2s[:], w2_dram)
            nc.sync.dma_start(w3s[:], w3_dram)
            nc.sync.dma_start(b1s[:], b1_dram)
            nc.sync.dma_start(b2s[:], b2_dram)
            nc.sync.dma_start(b3s[:], b3_dram)
            nc.vector.memset(ones[:], 1.0)

            # evac engine round-robin between the two PSUM readers
            evac_state = [0]

            def evac_engine():
                evac_state[0] ^= 1
                return evac_state[0]

            for it in range(NTILES):
                b0 = it * TILE
                # ---- cast pre-pass: x fp32 -> xbf bf16 (DRAM, batch-major) ----
                for c in range(NCHUNK):
                    xb = xbp.tile([128, J * D], bf16, tag="xb", name="xb")
                    nc.gpsimd.dma_start(
                        xb[:], x_flat[b0 + 128 * c: b0 + 128 * (c + 1), :])
                    nc.gpsimd.dma_start(
                        xbf[b0 + 128 * c: b0 + 128 * (c + 1), :], xb[:])
                # ---- feature-major tiles via DRAM-source xbar transpose ----
                xT = []
                for p in range(NPAIRS):
                    xt = xtp.tile([128, TILE], bf16, tag=f"xt{p}", name=f"xt{p}")
                    src_col = 128 * p if p < 10 else J * D - 128
                    nc.sync.dma_start(
                        xt[:], xbf[b0:b0 + TILE, src_col:src_col + 128],
                        transpose=True)
                    xT.append(xt)

                # ---- L1 / L2 per target ----
                h2tiles = []
                for t in range(21):
                    chunks = CHUNK_PLAN[t]
                    psum1 = ps1.tile([128, TILE], f32, tag="psum1", name="psum1")
                    for h in range(TILE // 512):
                        for ci, ch in enumerate(chunks):
                            col = CHUNK_COLS[(t, ci)]
                            nc.tensor.matmul(
                                psum1[:, 512 * h:512 * (h + 1)],
                                w1s[:, col:col + 128],
                                xT[ch["tile"]][:, 512 * h:512 * (h + 1)],
                                start=(ci == 0), stop=(ci == len(chunks) - 1))
                    h1 = actp.tile([128, TILE], bf16, tag="h1", name="h1")
                    nc.scalar.activation(h1[:], psum1[:], Relu,
                                         bias=b1s[:, t:t + 1], scale=1.0)

                    psum2 = ps2.tile([128, TILE], f32, tag="psum2", name="psum2")
                    for h in range(TILE // 512):
                        nc.tensor.matmul(
                            psum2[:, 512 * h:512 * (h + 1)],
                            w2s[:, 128 * t:128 * (t + 1)],
                            h1[:, 512 * h:512 * (h + 1)],
                            start=True, stop=True)
                    h2 = h2p.tile([128, TILE], bf16, tag=f"h2_{t}", name=f"h2_{t}")
                    nc.vector.tensor_scalar(
                        h2[:], psum2[:], b2s[:, t:t + 1], 0.0, Alu.add, Alu.max)
                    h2tiles.append(h2)

                # ---- L3: batch-major output, stationary = h2 chunks ----
                stg = stgp.tile([128, NCHUNK * J * D], f32, tag="stg", name="stg")
                stg3 = stg.rearrange("p (c f) -> p c f", f=J * D)
                for c in range(NCHUNK):
                    for gi, grp in enumerate(L3_GROUPS):
                        gw = 64 * len(grp)
                        psum3 = ps3.tile([128, 512], f32, tag="psum3", name="psum3")
                        nc.tensor.matmul(
                            psum3[:, 0:gw], ones[:, 0:128],
                            b3s[:, 64 * grp[0]:64 * grp[0] + gw],
                            start=True, stop=False, skip_group_check=True)
                        for k, t in enumerate(grp):
                            nc.tensor.matmul(
                                psum3[:, 64 * k:64 * (k + 1)],
                                h2tiles[t][:, 128 * c:128 * (c + 1)],
                                w3s[:, 64 * t:64 * (t + 1)],
                                start=False, stop=(k == len(grp) - 1),
                                skip_group_check=True)
                        dst = stg3[:, c, 64 * grp[0]:64 * grp[0] + gw]
                        if evac_engine():
                            nc.scalar.copy(dst, psum3[:, 0:gw])
                        else:
                            nc.vector.tensor_copy(dst, psum3[:, 0:gw])

                for c in range(NCHUNK):
                    nc.sync.dma_start(
                        out_flat[b0 + 128 * c: b0 + 128 * (c + 1), :],
                        stg3[:, c, :])

    nc.compile()
    return nc


PACKED = None
CHUNK_COLS = None
_NC = None
LAST_RESULT = None


def prepare(inputs):
    """Build (once) the bass module and the per-core input maps."""
    global PACKED, CHUNK_COLS, _NC
    import sys
    if "/opt/trn_rl_repo" not in sys.path:
        sys.path.insert(0, "/opt/trn_rl_repo")
    x = np.ascontiguousarray(np.asarray(inputs["x"], np.float32))
    PACKED, CHUNK_COLS = pack_weights(inputs)
    if _NC is None:
        _NC = build_bass_kernel()
    in_maps = []
    for core in range(NCORES):
        m = dict(PACKED)
        m["x"] = x[core * BC:(core + 1) * BC]
        in_maps.append(m)
    return _NC, in_maps


def kernel(**inputs):
    global LAST_RESULT
    nc, in_maps = prepare(inputs)
    from concourse.bass_utils import run_bass_kernel_spmd
    res = run_bass_kernel_spmd(nc, in_maps, core_ids=list(range(NCORES)),
                               tmpdir=os.environ.get("BASS_TMPDIR"))
    LAST_RESULT = res
    out = np.concatenate([r["out"] for r in res.results], 0)
    return out.reshape(B, J, D).astype(np.float32)



# revision 9
# speedup vs baseline: 1.2079x; 1.2079x over previous
"""Trainium2 Bass kernel for the 21-joint hand-graph message-passing MLP.

Math (per sample b, per target joint t with neighbor list S_t of length n):
    g   = concat(x[b, S_t[0]], ..., x[b, S_t[n-1]])          # [n*64]
    h1  = relu(g @ W1_t + b1_t)                              # [128]
    h2  = relu(h1 @ W2_t + b2_t)                             # [128]
    out[b, t] = h2 @ W3_t + b3_t                             # [64]

Strategy (pure data parallel over 8 NeuronCores, B=65536 -> 8192/core):
  - x is staged per batch-tile through a rotating bf16 DRAM buffer (SWDGE
    cast DMA), then DMA-xbar-transposed into 15 feature-major pair tiles
    xT[a] = [128 feats of nodes (a, a+1), batch], a in TILE_STARTS.  Using
    overlapping adjacent pairs (not just even pairs) lets every target pack
    its neighbor list into at most ceil(n/2)+1 K=128 contraction chunks:
    47 total vs 55 for even-pair-only packing.
  - L1/L2 run weight-stationary: psum[h, batch] = W.T @ act; relu+bias is
    fused into the PSUM->SBUF evacuation, explicitly balanced between
    ScalarE (activation) and VectorE (tensor_scalar) - the only two PSUM
    readers.
  - L3 runs activation-stationary (lhsT = h2 chunk [128 feats, 128 batch],
    rhs = W3 [128, 64]) so the output lands batch-major in PSUM.  b3 (a
    free-dim bias in this orientation) is added during the PSUM->SBUF
    evacuation against a host-replicated broadcast tile - no K=1 bias
    matmuls on the PE.
  - Output is staged bf16 and stored with two large DMAs per tile on the
    Scalar HWDGE queue (transposes own the Sync queue); the host upcasts
    to fp32.
"""

import os
import numpy as np
import ml_dtypes

B, J, D, H1, H2 = 65536, 21, 64, 128, 128
NCORES = 8
BC = B // NCORES          # 8192 samples per core
TILE = 1024               # batch tile (2 PSUM banks wide in fp32)
NTILES = BC // TILE       # 8
NCHUNK = TILE // 128      # 8 L3 batch chunks of 128 per tile

FINGER_BASE = [4 * f + 1 for f in range(5)]
NEIGH = {
    6: [[0, 1, 5, 9, 13, 17]],
    5: [[0, 5, 6, 1, 9], [0, 9, 10, 5, 13], [0, 13, 14, 9, 17]],
    4: [[0, 1, 2, 5], [0, 17, 18, 13]],
    3: [r for b in FINGER_BASE for r in ([b, b + 1, b + 2], [b + 1, b + 2, b + 3])],
    2: [[b + 2, b + 3] for b in FINGER_BASE],
}
OUT = {
    6: [0],
    5: [5, 9, 13],
    4: [1, 17],
    3: [j for b in FINGER_BASE for j in (b + 1, b + 2)],
    2: [b + 3 for b in FINGER_BASE],
}
GROUPS = [6, 5, 4, 3, 2]

# target t -> (n, row index within its group, neighbor list)
TARGET = {}
for n in GROUPS:
    for row, t in enumerate(OUT[n]):
        TARGET[t] = (n, row, list(NEIGH[n][row]))

# xT pair tiles: tile `a` holds nodes (a, a+1) feature-major; built by one
# DMA-xbar transpose of xbf columns [64a : 64a+128].
TILE_STARTS = [0, 2, 3, 5, 6, 7, 9, 10, 11, 13, 14, 15, 17, 18, 19]


def node_slots(j):
    """All (tile, half) positions where node j lives."""
    out = []
    if j in TILE_STARTS:
        out.append((j, 0))
    if j - 1 in TILE_STARTS:
        out.append((j - 1, 1))
    return out


def build_chunk_plan():
    """Per target, split neighbor positions into K=128 contraction chunks.

    chunk = dict(tile, slots) with slots = (pos_or_None for half 0,
    pos_or_None for half 1); position i covers W1 rows 64*i : 64*i+64.
    Adjacent neighbors (j, j+1) share a chunk via pair tile j; leftovers
    become half-empty chunks on any tile containing their node (the unused
    64 lhsT rows are zero in the packed W1, so any tile works).
    """
    plan = {}
    for t in range(21):
        n, _, S = TARGET[t]
        best = None
        # brute-force max matching over adjacent pairs (n <= 6)
        import itertools
        idx = list(range(n))
        pairs = [(i, k) for i in idx for k in idx if i < k
                 and abs(S[i] - S[k]) == 1 and min(S[i], S[k]) in TILE_STARTS]

        def search(used, chosen):
            nonlocal best
            cand = [p for p in pairs if not (used & (1 << p[0])) and not (used & (1 << p[1]))]
            if not cand:
                if best is None or len(chosen) > len(best):
                    best = list(chosen)
                return
            for p in cand:
                search(used | (1 << p[0]) | (1 << p[1]), chosen + [p])
            if best is None or len(chosen) > len(best):
                best = list(chosen)

        search(0, [])
        chunks = []
        used = set()
        for i, k in best:
            a, b = S[i], S[k]
            lo = min(a, b)
            pi, pk = (i, k) if a == lo else (k, i)
            chunks.append(dict(tile=lo, slots=(pi, pk)))
            used.update((i, k))
        for i in range(n):
            if i in used:
                continue
            tile_a, half = node_slots(S[i])[0]
            slots = (i, None) if half == 0 else (None, i)
            chunks.append(dict(tile=tile_a, slots=slots))
        plan[t] = chunks
    return plan


CHUNK_PLAN = build_chunk_plan()
TOTAL_CHUNKS = sum(len(v) for v in CHUNK_PLAN.values())

# L3 node groups sharing one PSUM bank (64 fp32 of output each)
L3_GROUPS = [list(range(0, 8)), list(range(8, 16)), list(range(16, 21))]


def pack_weights(inputs):
    """Host-side prep: permute/pack all weights into a handful of flat arrays."""
    bf16 = ml_dtypes.bfloat16
    w1p = np.zeros((128, 128 * TOTAL_CHUNKS), np.float32)
    col = 0
    chunk_cols = {}
    for t in range(21):
        n, row, S = TARGET[t]
        W1 = np.asarray(inputs[f"w1_g{n}"][row], np.float32)  # [n*64, 128]
        for ci, ch in enumerate(CHUNK_PLAN[t]):
            for half, pos in enumerate(ch["slots"]):
                if pos is not None:
                    w1p[64 * half:64 * half + 64, col:col + 128] = \
                        W1[64 * pos:64 * pos + 64]
            chunk_cols[(t, ci)] = col
            col += 128
    w2p = np.zeros((128, 128 * 21), np.float32)
    w3p = np.zeros((128, 64 * 21), np.float32)
    b1p = np.zeros((128, 21), np.float32)
    b2p = np.zeros((128, 21), np.float32)
    b3row = np.zeros(64 * 21, np.float32)
    for t in range(21):
        n, row, _ = TARGET[t]
        w2p[:, 128 * t:128 * (t + 1)] = np.asarray(inputs[f"w2_g{n}"][row])
        w3p[:, 64 * t:64 * (t + 1)] = np.asarray(inputs[f"w3_g{n}"][row])
        b1p[:, t] = np.asarray(inputs[f"b1_g{n}"][row])
        b2p[:, t] = np.asarray(inputs[f"b2_g{n}"][row])
        b3row[64 * t:64 * t + 64] = np.asarray(inputs[f"b3_g{n}"][row])
    b3bc = np.ascontiguousarray(np.broadcast_to(b3row, (128, 64 * 21)))
    return dict(
        w1p=w1p.astype(bf16), w2p=w2p.astype(bf16), w3p=w3p.astype(bf16),
        b1p=b1p, b2p=b2p, b3bc=b3bc,
    ), chunk_cols


def numpy_emulate(inputs, x):
    """Bit-layout-faithful numpy model of what the HW kernel computes (minus
    PSUM rounding): used to validate the chunk plan / packing offline."""
    bf16 = ml_dtypes.bfloat16
    packed, chunk_cols = pack_weights(inputs)
    xb = x.astype(bf16)  # [Bn, 21, 64]
    Bn = x.shape[0]
    xT = {}
    for a in TILE_STARTS:
        xT[a] = np.concatenate([xb[:, a], xb[:, a + 1]], 1).T  # [128, Bn]
    out = np.zeros((Bn, 21, 64), np.float32)
    for t in range(21):
        psum1 = np.zeros((128, Bn), np.float32)
        for ci, ch in enumerate(CHUNK_PLAN[t]):
            col = chunk_cols[(t, ci)]
            lhsT = packed["w1p"][:, col:col + 128].astype(np.float32)
            rhs = xT[ch["tile"]].astype(np.float32)
            psum1 += lhsT.T @ rhs
        h1 = np.maximum(psum1 + packed["b1p"][:, t:t + 1], 0).astype(bf16)
        w2 = packed["w2p"][:, 128 * t:128 * (t + 1)].astype(np.float32)
        psum2 = w2.T @ h1.astype(np.float32)
        h2 = np.maximum(psum2 + packed["b2p"][:, t:t + 1], 0).astype(bf16)
        w3 = packed["w3p"][:, 64 * t:64 * (t + 1)].astype(np.float32)
        b3 = packed["b3bc"][0, 64 * t:64 * t + 64]
        o = (h2.astype(np.float32).T @ w3) + b3[None, :]
        out[:, t] = o.astype(bf16).astype(np.float32)
    return out


# ---------------------------------------------------------------------------
# Bass kernel
# ---------------------------------------------------------------------------

def build_bass_kernel():
    import concourse.bass as bass
    import concourse.tile as tile
    from concourse import bacc, mybir

    bf16 = mybir.dt.bfloat16
    f32 = mybir.dt.float32
    Relu = mybir.ActivationFunctionType.Relu
    Alu = mybir.AluOpType

    nc = bacc.Bacc("TRN2", target_bir_lowering=False, debug=False,
                   num_devices=NCORES)
    x_dram = nc.dram_tensor("x", [BC, J, D], f32, kind="ExternalInput").ap()
    out_dram = nc.dram_tensor("out", [BC, J, D], bf16, kind="ExternalOutput").ap()
    w1_dram = nc.dram_tensor("w1p", [128, 128 * TOTAL_CHUNKS], bf16,
                             kind="ExternalInput").ap()
    w2_dram = nc.dram_tensor("w2p", [128, 128 * 21], bf16, kind="ExternalInput").ap()
    w3_dram = nc.dram_tensor("w3p", [128, 64 * 21], bf16, kind="ExternalInput").ap()
    b1_dram = nc.dram_tensor("b1p", [128, 21], f32, kind="ExternalInput").ap()
    b2_dram = nc.dram_tensor("b2p", [128, 21], f32, kind="ExternalInput").ap()
    b3_dram = nc.dram_tensor("b3bc", [128, 64 * 21], f32, kind="ExternalInput").ap()

    F = J * D  # 1344
    # [128, BC//128, F] views: global batch row = q*128 + p
    x_q = x_dram.rearrange("(q p) t d -> p q (t d)", p=128)
    out_q = out_dram.rearrange("(q p) t d -> p q (t d)", p=128)

    with tile.TileContext(nc) as tc:
        with (
            tc.tile_pool(name="wpool", bufs=1) as wpool,
            tc.tile_pool(name="xbp", bufs=2) as xbp,
            tc.tile_pool(name="xtp", bufs=2) as xtp,
            tc.tile_pool(name="actp", bufs=3) as actp,
            tc.tile_pool(name="h2p", bufs=1) as h2p,
            tc.tile_pool(name="stgp", bufs=1) as stgp,
            tc.tile_pool(name="ps1", bufs=2, space="PSUM") as ps1,
            tc.tile_pool(name="ps2", bufs=1, space="PSUM") as ps2,
            tc.tile_pool(name="ps3", bufs=2, space="PSUM") as ps3,
            tc.tile_pool(name="dramp", bufs=3, space="DRAM") as dramp,
        ):
            w1s = wpool.tile([128, 128 * TOTAL_CHUNKS], bf16, name="w1s")
            w2s = wpool.tile([128, 128 * 21], bf16, name="w2s")
            w3s = wpool.tile([128, 64 * 21], bf16, name="w3s")
            b1s = wpool.tile([128, 21], f32, name="b1s")
            b2s = wpool.tile([128, 21], f32, name="b2s")
            b3s = wpool.tile([128, 64 * 21], f32, name="b3s")
            nc.sync.dma_start(w1s[:], w1_dram)
            nc.sync.dma_start(w2s[:], w2_dram)
            nc.sync.dma_start(w3s[:], w3_dram)
            nc.sync.dma_start(b1s[:], b1_dram)
            nc.sync.dma_start(b2s[:], b2_dram)
            nc.sync.dma_start(b3s[:], b3_dram)

            # explicit PSUM-evac engine balancing: the two PSUM readers
            # (ScalarE 1.2GHz, VectorE 0.96GHz) get ops by tracked cost
            eng_load = [0.0, 0.0]  # [scalar_ns, vector_ns]

            def evac_relu(dst, src, bias_col, ncols):
                if eng_load[0] + (ncols + 352) / 1.2 <= \
                   eng_load[1] + (ncols + 250) / 0.96:
                    eng_load[0] += (ncols + 352) / 1.2
                    nc.scalar.activation(dst, src, Relu, bias=bias_col, scale=1.0)
                else:
                    eng_load[1] += (ncols + 250) / 0.96
                    nc.vector.tensor_scalar(dst, src, bias_col, 0.0,
                                            Alu.add, Alu.max)

            def evac_add(dst, src, bcast, ncols):
                # free-dim bias add: tensor_tensor exists only on VectorE
                eng_load[1] += (ncols + 250) / 0.96
                nc.vector.tensor_tensor(dst, src, bcast, Alu.add)

            for it in range(NTILES):
                b0 = it * TILE
                # ---- cast pre-pass: x fp32 -> bf16 DRAM staging (rotating
                # per-tile buffer; region-disjoint so tiles pipeline) ----
                xbf = dramp.tile([TILE, F], bf16, tag="xbf", name="xbf")
                xbf3 = xbf.rearrange("(c p) f -> p c f", p=128)
                for c in range(2):
                    xb = xbp.tile([128, 4, F], bf16, tag="xb", name="xb")
                    nc.gpsimd.dma_start(
                        xb[:], x_q[:, NCHUNK * it + 4 * c:
                                   NCHUNK * it + 4 * (c + 1), :])
                    nc.gpsimd.dma_start(xbf3[:, 4 * c:4 * (c + 1), :], xb[:])

                # ---- feature-major pair tiles via DRAM-source xbar transpose
                xT = {}
                for a in TILE_STARTS:
                    xt = xtp.tile([128, TILE], bf16, tag=f"xt{a}", name=f"xt{a}")
                    nc.sync.dma_start(xt[:], xbf[:, 64 * a:64 * a + 128],
                                      transpose=True)
                    xT[a] = xt

                # ---- L1 / L2 per target ----
                h2tiles = []
                for t in range(21):
                    chunks = CHUNK_PLAN[t]
                    psum1 = ps1.tile([128, TILE], f32, tag="psum1", name="psum1")
                    for ci, ch in enumerate(chunks):
                        col = CHUNK_COLS[(t, ci)]
                        for h in range(TILE // 512):
                            nc.tensor.matmul(
                                psum1[:, 512 * h:512 * (h + 1)],
                                w1s[:, col:col + 128],
                                xT[ch["tile"]][:, 512 * h:512 * (h + 1)],
                                start=(ci == 0), stop=(ci == len(chunks) - 1),
                                skip_group_check=True)
                    h1 = actp.tile([128, TILE], bf16, tag="h1", name="h1")
                    evac_relu(h1[:], psum1[:], b1s[:, t:t + 1], TILE)

                    psum2 = ps2.tile([128, TILE], f32, tag="psum2", name="psum2")
                    for h in range(TILE // 512):
                        nc.tensor.matmul(
                            psum2[:, 512 * h:512 * (h + 1)],
                            w2s[:, 128 * t:128 * (t + 1)],
                            h1[:, 512 * h:512 * (h + 1)],
                            start=True, stop=True)
                    h2 = h2p.tile([128, TILE], bf16, tag=f"h2_{t}", name=f"h2_{t}")
                    evac_relu(h2[:], psum2[:], b2s[:, t:t + 1], TILE)
                    h2tiles.append(h2)

                # ---- L3: batch-major output, stationary = h2 chunks ----
                stg = stgp.tile([128, NCHUNK * F], bf16, tag="stg", name="stg")
                stg3 = stg.rearrange("p (c f) -> p c f", f=F)
                for c in range(NCHUNK):
                    for gi, grp in enumerate(L3_GROUPS):
                        gw = 64 * len(grp)
                        psum3 = ps3.tile([128, 512], f32, tag="psum3", name="psum3")
                        for k, t in enumerate(grp):
                            nc.tensor.matmul(
                                psum3[:, 64 * k:64 * (k + 1)],
                                h2tiles[t][:, 128 * c:128 * (c + 1)],
                                w3s[:, 64 * t:64 * (t + 1)],
                                start=True, stop=True,
                                skip_group_check=True)
                        c0 = 64 * grp[0]
                        evac_add(stg3[:, c, c0:c0 + gw], psum3[:, 0:gw],
                                 b3s[:, c0:c0 + gw], gw)

                # ---- store: two large DMAs on the Scalar HWDGE queue ----
                for c in range(2):
                    nc.scalar.dma_start(
                        out_q[:, NCHUNK * it + 4 * c:
                              NCHUNK * it + 4 * (c + 1), :],
                        stg3[:, 4 * c:4 * (c + 1), :])

    nc.compile()
    return nc


PACKED = None
CHUNK_COLS = None
_NC = None
LAST_RESULT = None


def prepare(inputs):
    """Build (once) the bass module and the per-core input maps."""
    global PACKED, CHUNK_COLS, _NC
    import sys
    if "/opt/trn_rl_repo" not in sys.path:
        sys.path.insert(0, "/opt/trn_rl_repo")
    x = np.ascontiguousarray(np.asarray(inputs["x"], np.float32))
    PACKED, CHUNK_COLS = pack_weights(inputs)
    if _NC is None:
        _NC = build_bass_kernel()
    in_maps = []
    for core in range(NCORES):
        m = dict(PACKED)
        m["x"] = x[core * BC:(core + 1) * BC]
        in_maps.append(m)
    return _NC, in_maps


def kernel(**inputs):
    global LAST_RESULT
    nc, in_maps = prepare(inputs)
    from concourse.bass_utils import run_bass_kernel_spmd
    res = run_bass_kernel_spmd(nc, in_maps, core_ids=list(range(NCORES)),
                               tmpdir=os.environ.get("BASS_TMPDIR"))
    LAST_RESULT = res
    out = np.concatenate([r["out"] for r in res.results], 0)
    return out.reshape(B, J, D).astype(np.float32)
